# revision 18
# baseline (speedup 1.0000x reference)
"""Sparse L1-distance attention (nn_L1AttnSparse) on 8 Trainium2 NeuronCores.

Layout strategy (v2): dst tokens are split across the 8 cores (256 each).
k/v tables are stored fp16 with a host-side feature permutation so that the
transpose-mode SWDGE gather (dma_gather transpose=True) lands features on
partitions with head h = partition//16 constant per partition.  The L1
score reduction over the 64 head features then becomes a PE matmul with a
constant 0/1 block mask (accumulated over the 4 column-chunks), softmax
needs no max-subtraction (scores <= 0; a constant bias keeps exp() in fp16
range, cancelled by the normalizer), and the weighted v-sum runs as fp16
tensor_tensor ops (2x DVE mode) with a tree reduction over slots.
"""

import sys

sys.path.insert(0, "/opt/trn_rl_repo")

import numpy as np

import concourse.bass as bass
import concourse.tile as tile
from concourse import bacc, mybir
from concourse.bass_utils import run_bass_kernel_spmd

BS = 2
N_TOK = 2048
NH = 8
W = 64
S = 32  # dst_mxlen
HW = NH * W  # 512 features per (b, tok) row
N_CORES = 8
DT = N_TOK // N_CORES  # dst tokens per core = 256
CHUNKS = DT // 128  # dst chunks of 128 per core = 2
NB = BS * CHUNKS  # blocks per core = 4
SHALF = S // 2  # slots per gather half = 16
EDGES_H = SHALF * 128  # edges per gather = 2048
C4 = HW // 128  # feature column-chunks in transpose-gather = 4
CEXP = 40.0  # constant score bias: exp((CEXP - L)/8), cancels in normalize
SCALE = 1.0 / np.sqrt(W)  # 1/8

# feature permutation: table column pos = c*128 + p holds original feature
# h*64 + c*16 + r where p = h*16 + r  ->  head h == p//16 for every c.
_P = np.arange(128)
_C = np.arange(C4)
COLPERM = (
    (_P[None, :] // 16) * 64 + _C[:, None] * 16 + (_P[None, :] % 16)
).reshape(-1)  # [pos] -> original feature index


def _wrap_idx(flat):
    """int16 index list -> [128, n/16] tile layout: idx i at [i%16, i//16],
    replicated down the 8 groups of 16 partitions."""
    n = flat.shape[0]
    w16 = np.zeros((16, n // 16), dtype=np.int16)
    w16[np.arange(n) % 16, np.arange(n) // 16] = flat
    return np.tile(w16, (8, 1))


def build_kernel():
    nc = bacc.Bacc(
        "TRN2", target_bir_lowering=False, debug=False, num_devices=N_CORES,
        dynamic_dma_scratch_size=32768, num_swdge_queues=1,
    )
    f16 = mybir.dt.float16
    f32 = mybir.dt.float32
    i16 = mybir.dt.int16

    kf = nc.dram_tensor("kf", [BS * N_TOK, HW], f16, kind="ExternalInput").ap()
    vf = nc.dram_tensor("vf", [BS * N_TOK, HW], f16, kind="ExternalInput").ap()
    qT = nc.dram_tensor("qT", [NB, 128, C4 * 128], f16, kind="ExternalInput").ap()
    msk = nc.dram_tensor("msk", [128, 128], f16, kind="ExternalInput").ap()
    idx = nc.dram_tensor(
        "idx", [NB, 2, 128, EDGES_H // 16], i16, kind="ExternalInput"
    ).ap()
    oc = nc.dram_tensor("oc", [NB, 128, C4 * 128], f16, kind="ExternalOutput").ap()

    with tile.TileContext(nc) as tc:
        with (
            nc.allow_low_precision(reason="fp16 datapath, fp32 score accum"),
            tc.tile_pool(name="kgp", bufs=2) as kgp,
            tc.tile_pool(name="vgp", bufs=2) as vgp,
            tc.tile_pool(name="small", bufs=2) as smp,
            tc.tile_pool(name="const", bufs=1) as cst,
            tc.tile_pool(name="psum", bufs=2, space="PSUM") as psp,
        ):
            msk_t = cst.tile([128, 128], f16, tag="msk")
            bias_t = cst.tile([128, 1], f32, tag="bias")

            QTR = EDGES_H // 2  # 1024 gathered rows per quarter

            def make_inputs_k(blk):
                st = {}
                idx_ts = []
                for hf in range(2):
                    it = smp.tile([128, EDGES_H // 16], i16, tag=f"idx{hf}")
                    nc.sync.dma_start(out=it[:], in_=idx[blk, hf])
                    idx_ts.append(it)
                qt = smp.tile([128, C4, 128], f16, tag="qt")
                nc.sync.dma_start(
                    out=qt[:], in_=qT[blk].rearrange("p (c d) -> p c d", c=C4)
                )
                # k gathered in slot-block quarters (1024 rows) so the score
                # pipeline can start on the first quarter early
                kgs = []
                for qq in range(4):
                    kg = kgp.tile([128, C4, QTR], f16, tag=f"kg{qq}")
                    it = idx_ts[qq // 2]
                    nc.gpsimd.dma_gather(
                        kg[:], kf, it[:, (qq % 2) * 64 : (qq % 2 + 1) * 64],
                        QTR, QTR, HW,
                        transpose=True, queue_num=0,
                    )
                    kgs.append(kg)
                st["qt"], st["kgs"], st["idx"] = qt, kgs, idx_ts
                return st

            def make_inputs_v(st):
                idx_ts = st["idx"]
                vgs = []
                for qq in range(4):
                    vg = vgp.tile([128, C4, QTR], f16, tag=f"vg{qq}")
                    it = idx_ts[qq // 2]
                    nc.gpsimd.dma_gather(
                        vg[:], vf, it[:, (qq % 2) * 64 : (qq % 2 + 1) * 64],
                        QTR, QTR, HW,
                        transpose=True, queue_num=0,
                    )
                    vgs.append(vg)
                st["vgs"] = vgs

            def emit_score(blk, st):
                qt, kgs = st["qt"], st["kgs"]
                E16 = smp.tile([128, S, 128], f16, tag="E")
                st["E16"] = E16
                for qq in range(4):
                    kg4 = kgs[qq][:].rearrange("p c (s d) -> p c s d", d=128)
                    # kg <- kg - q (broadcast over slots); fp16 2x mode
                    nc.vector.tensor_tensor(
                        out=kg4, in0=kg4,
                        in1=qt[:, :, None, :].to_broadcast([128, C4, 8, 128]),
                        op=mybir.AluOpType.subtract,
                    )
                    # |diff| on the Activation engine
                    nc.scalar.activation(
                        out=kg4, in_=kg4,
                        func=mybir.ActivationFunctionType.Abs,
                    )
                    # L via PE: psum[x, (s,d)] = sum_c sum_p msk[p,x]*|diff|
                    ps = psp.tile([128, 8, 128], f32, tag="ps")
                    for half in range(2):
                        out_sl = ps[:, half * 4 : (half + 1) * 4, :]
                        s0 = half * 4
                        for c in range(C4):
                            nc.tensor.matmul(
                                out_sl, msk_t[:], kg4[:, c, s0 : s0 + 4, :],
                                start=(c == 0), stop=(c == C4 - 1),
                            )
                    # E = exp((CEXP - L)/8), fp16, replicated over 16-groups
                    nc.scalar.activation(
                        out=E16[:, qq * 8 : (qq + 1) * 8, :], in_=ps[:],
                        func=mybir.ActivationFunctionType.Exp,
                        scale=-SCALE, bias=bias_t[:],
                    )

            def emit_weight(blk, st):
                E16, vgs = st["E16"], st["vgs"]
                # denominator: tree-sum E over slots (fp16 TT adds, 2x mode)
                dtr = smp.tile([128, 16, 128], f16, tag="dtr")
                nc.vector.tensor_tensor(
                    out=dtr[:], in0=E16[:, :16, :], in1=E16[:, 16:, :],
                    op=mybir.AluOpType.add,
                )
                n = 8
                while n >= 2:
                    nc.vector.tensor_tensor(
                        out=dtr[:, :n, :], in0=dtr[:, :n, :],
                        in1=dtr[:, n : 2 * n, :],
                        op=mybir.AluOpType.add,
                    )
                    n //= 2
                den = smp.tile([128, 128], f32, tag="den")
                nc.vector.tensor_tensor(
                    out=den[:], in0=dtr[:, 0, :], in1=dtr[:, 1, :],
                    op=mybir.AluOpType.add,
                )
                rden = smp.tile([128, 128], f16, tag="rden")
                nc.vector.reciprocal(rden[:], den[:])
                # weighted v: vg *= E (broadcast over c), then tree-sum the
                # slots, first across quarter-tile pairs then within tiles
                v4 = [
                    vgs[qq][:].rearrange("p c (s d) -> p c s d", d=128)
                    for qq in range(4)
                ]
                for qq in range(4):
                    nc.vector.tensor_tensor(
                        out=v4[qq], in0=v4[qq],
                        in1=E16[:, None, qq * 8 : (qq + 1) * 8, :]
                        .to_broadcast([128, C4, 8, 128]),
                        op=mybir.AluOpType.mult,
                    )
                for qq in (0, 2):
                    nc.vector.tensor_tensor(
                        out=v4[qq], in0=v4[qq], in1=v4[qq + 1],
                        op=mybir.AluOpType.add,
                    )
                    n = 4
                    while n >= 1:
                        nc.vector.tensor_tensor(
                            out=v4[qq][:, :, :n, :], in0=v4[qq][:, :, :n, :],
                            in1=v4[qq][:, :, n : 2 * n, :],
                            op=mybir.AluOpType.add,
                        )
                        n //= 2
                vsum = smp.tile([128, C4, 128], f16, tag="vsum")
                nc.vector.tensor_tensor(
                    out=vsum[:], in0=v4[0][:, :, 0, :], in1=v4[2][:, :, 0, :],
                    op=mybir.AluOpType.add,
                )
                ot = smp.tile([128, C4, 128], f16, tag="ot")
                nc.vector.tensor_tensor(
                    out=ot[:], in0=vsum[:],
                    in1=rden[:, None, :].to_broadcast([128, C4, 128]),
                    op=mybir.AluOpType.mult,
                )
                # store on the ACT engine's DGE so SP's in-order queue never
                # delays the next block's idx/q loads behind this store
                nc.scalar.dma_start(
                    out=oc[blk].rearrange("p (c d) -> p c d", c=C4), in_=ot[:]
                )

            # software pipeline: k-gathers run two blocks ahead and v-gathers
            # one block ahead on the serialized DMA engines; block N+1's score
            # phase is emitted before block N's weighting phase so DVE fills
            # the softmax latency with the next block's subtractions
            pend = {0: make_inputs_k(0)}
            nc.gpsimd.memset(bias_t[:], CEXP * SCALE)
            nc.sync.dma_start(out=msk_t[:], in_=msk)
            pend[1] = make_inputs_k(1)
            make_inputs_v(pend[0])
            emit_score(0, pend[0])
            for blk in range(NB):
                if blk + 1 < NB:
                    make_inputs_v(pend[blk + 1])
                if blk + 2 < NB:
                    pend[blk + 2] = make_inputs_k(blk + 2)
                if blk + 1 < NB:
                    emit_score(blk + 1, pend[blk + 1])
                emit_weight(blk, pend.pop(blk))
    nc.compile()
    return nc


_NC_CACHE = None


def kernel(v, q, k, coo, dst_mxlen):
    global _NC_CACHE
    assert int(dst_mxlen) == S
    v = np.asarray(v, dtype=np.float32)
    q = np.asarray(q, dtype=np.float32)
    k = np.asarray(k, dtype=np.float32)
    coo = np.asarray(coo)

    # src table: srct[t, s] = src index of edge (dst=t, slot=s)
    srct = np.zeros((N_TOK, S), dtype=np.int64)
    srct[coo[:, 0], coo[:, 2]] = coo[:, 1]

    kf = k.reshape(BS * N_TOK, HW)[:, COLPERM].astype(np.float16)
    vf = v.reshape(BS * N_TOK, HW)[:, COLPERM].astype(np.float16)
    q2 = q.reshape(BS, N_TOK, HW)[:, :, COLPERM].astype(np.float16)
    mskh = np.zeros((128, 128), dtype=np.float16)
    mskh[np.arange(128)[:, None] // 16 == np.arange(128)[None, :] // 16] = 1.0

    if _NC_CACHE is None:
        _NC_CACHE = build_kernel()
    nc = _NC_CACHE

    in_maps = []
    for core in range(N_CORES):
        lo0 = core * DT
        qTh = np.empty((NB, 128, C4 * 128), dtype=np.float16)
        idxh = np.empty((NB, 2, 128, EDGES_H // 16), dtype=np.int16)
        for b in range(BS):
            for c in range(CHUNKS):
                blk = b * CHUNKS + c
                lo = lo0 + c * 128
                # [d, pos] -> [p, c4, d]
                slab = q2[b, lo : lo + 128].reshape(128, C4, 128)
                qTh[blk] = slab.transpose(2, 1, 0).reshape(128, C4 * 128)
                for hf in range(2):
                    sl = slice(hf * SHALF, (hf + 1) * SHALF)
                    flat = (b * N_TOK + srct[lo : lo + 128, sl].T).reshape(-1)
                    idxh[blk, hf] = _wrap_idx(flat.astype(np.int16))
        in_maps.append(
            {"kf": kf, "vf": vf, "qT": qTh, "msk": mskh, "idx": idxh}
        )

    res = run_bass_kernel_spmd(nc, in_maps, list(range(N_CORES)))
    out = np.empty((BS, N_TOK, HW), dtype=np.float32)
    for core in range(N_CORES):
        lo0 = core * DT
        for b in range(BS):
            for c in range(CHUNKS):
                blk = b * CHUNKS + c
                lo = lo0 + c * 128
                o3 = res.results[core]["oc"][blk].reshape(128, C4, 128)
                out[b, lo : lo + 128, COLPERM] = (
                    o3.transpose(1, 0, 2).reshape(C4 * 128, 128)
                )
    return out.reshape(BS, N_TOK, NH, W)
